# revision 17
# baseline (speedup 1.0000x reference)
"""Trainium2 Bass kernel for nn_BasicBlock (ShiftNet+AdderNet block), data-parallel on 8 cores.

Adder conv decomposition (per valid (ci,tap) term, exact up to level approx):
  -|p-w| = -|p| + cK + 2*delta*1[p<0] - 2*clamp_v(p)
where v = v(w) is one of 6 global value levels (1D k-means per sign, snapped
to the fp8e4 grid), delta = v - w, cK = w + 2|v|*1[w<0], and
  clamp_v(p) = relu(v - relu(p))       (v > 0)
  clamp'_v(p) = relu(|v| - relu(-p))   (v < 0)
Planes are tap-independent: per phase we build |p| (f32r), negind = tau*1[p<0]
(fp8), and 6 clamp planes (fp8). The level matmuls run as fp8 DoubleRow pairs
(two 128-deep contraction planes per instruction at 0.5 cyc/row); the delta
conv rides (negind, zero) as a 4th pair; |p| uses an all-(-1) f32r matmul.
Borders: H-pad taps are skipped by matmul row restriction; W-pad columns hold
plane(0) values; both corrected by a 9-class border table matmul.
BN uses one [C,3] AllReduce per BN (parallel variance combine).
"""
import numpy as np
import ml_dtypes

NCORES = 8
NSH = 8            # images per core
HALF = 4           # images per half-pass
H = W = 32
C = 128
WP = 34            # w-padded plane width
PLANE = HALF * H * WP
EPS = 1e-5
TAU = 2.0 ** -8
LPS = 3            # levels per sign
NLEV = 2 * LPS
NPR = NLEV // 2    # level pairs (the delta conv adds one more pair)

_CACHE = {}
F8NP = ml_dtypes.float8_e4m3


def _f8(x):
    return np.asarray(x, np.float64).astype(F8NP).astype(np.float64)


def _kmeans1d(vals, k, iters=50):
    cent = np.quantile(vals, (np.arange(k) + 0.5) / k)
    for _ in range(iters):
        idx = np.abs(vals[:, None] - cent[None, :]).argmin(1)
        for j in range(k):
            m = idx == j
            if m.any():
                cent[j] = vals[m].mean()
        cent = np.sort(cent)
    cent = np.unique(_f8(cent))
    if len(cent) < k:
        cent = np.concatenate([cent, np.repeat(cent[-1], k - len(cent))])
    return cent


def _host_prep_adder(wa64):
    """wa64: [co, ci, 3, 3] float64. Device arrays for one adder conv."""
    co_n = wa64.shape[0]
    wk = wa64.reshape(co_n, C, 9)                    # [co, ci, tap]
    assert (wk != 0.0).all(), "zero adder weight breaks sign split"
    lev_p = _kmeans1d(wk[wk > 0], LPS)               # ascending, > 0
    lev_n = -_kmeans1d(-wk[wk < 0], LPS)[::-1]       # ascending, < 0
    levels = np.concatenate([lev_p, lev_n])          # slots 0..2 pos, 3..5 neg
    ip = np.abs(wk[..., None] - lev_p[None, None, None, :]).argmin(-1)
    inn = np.abs(wk[..., None] - lev_n[None, None, None, :]).argmin(-1)
    pos = wk > 0
    assign = np.where(pos, ip, LPS + inn)            # [co, ci, tap]
    v = np.where(pos, lev_p[ip], lev_n[inn])
    delta = v - wk
    cK = wk + 2.0 * np.abs(v) * (~pos)
    cb = cK.sum(axis=(1, 2)).reshape(co_n, 1)        # [co, 1]
    # lhsT pairs: [ci, tap, pair, half, co] fp8
    #   slots 0..2 (pos, ACT +clamp) -> -2 ; slots 3..5 (neg, DVE -clamp) -> +2
    #   pair NPR: half0 = delta kernel 2*delta/TAU on negind, half1 = 0
    E = np.zeros((C, 9, NPR + 1, 2, co_n), np.float64)
    for lvl in range(NLEV):
        sgn = -2.0 if lvl < LPS else 2.0
        pr, hf = lvl // 2, lvl % 2
        for tap in range(9):
            sel = assign[:, :, tap] == lvl           # [co, ci]
            E[:, tap, pr, hf, :][sel.T] = sgn
    for tap in range(9):
        E[:, tap, NPR, 0, :] = (2.0 / TAU) * delta[:, :, tap].T
    # border table: 9 classes [9, co]
    absw = np.abs(wk)
    hcorr = -(absw + cK).sum(axis=1)                 # [co, tap]
    wcorr = (2.0 * delta * pos).sum(axis=1)          # [co, tap]
    btbl = np.zeros((9, co_n), np.float64)
    for hcls in range(3):
        for wcls in range(3):
            cls = hcls * 3 + wcls
            for tap in range(9):
                kh, kw = tap // 3, tap % 3
                h_pad = (hcls == 0 and kh == 0) or (hcls == 2 and kh == 2)
                w_pad = (wcls == 0 and kw == 0) or (wcls == 2 and kw == 2)
                if h_pad:
                    btbl[cls] += hcorr[:, tap]
                elif w_pad:
                    btbl[cls] += wcorr[:, tap]
    return dict(levels=levels.astype(np.float32),
                e=np.ascontiguousarray(E.astype(F8NP)),
                cb=cb.astype(np.float32), btbl=btbl.astype(np.float32))


def _host_mcls():
    """class-indicator rhs [9 cls, 4 parity, 8*34] (f32); cols 0/33 junk -> 0."""
    m = np.zeros((9, 4, 8, WP), np.float32)
    for par in range(4):
        for hr in range(8):
            h = par * 8 + hr
            hcls = 0 if h == 0 else (2 if h == 31 else 1)
            for w in range(W):
                wcls = 0 if w == 0 else (2 if w == 31 else 1)
                m[hcls * 3 + wcls, par, hr, w + 1] = 1.0
    return np.ascontiguousarray(m.reshape(9, 4, 8 * WP))


def _build_program(use_cc=True, debug=False):
    import concourse.bass as bass
    import concourse.bacc as bacc
    import concourse.tile as tile
    import contextlib
    from concourse import mybir

    F32 = mybir.dt.float32
    F32R = mybir.dt.float32r
    BF16 = mybir.dt.bfloat16
    F8 = mybir.dt.float8e4
    AT = mybir.ActivationFunctionType
    OP = mybir.AluOpType
    PM = mybir.MatmulPerfMode.DoubleRow

    nc = bacc.Bacc("TRN2", target_bir_lowering=False, debug=False,
                   num_devices=NCORES if use_cc else 1)

    x_ap = nc.dram_tensor("x", [NSH, C, H, W], F32, kind="ExternalInput").ap()
    gb_ap = nc.dram_tensor("gb", [C, 4], F32, kind="ExternalInput").ap()
    mcls_ap = nc.dram_tensor("mcls", [9, 4, 8 * WP], F32R, kind="ExternalInput").ap()
    lv_ap = nc.dram_tensor("lv", [C, 2 * NLEV], F32, kind="ExternalInput").ap()
    wshs, es_, cbs_, btbls = [], [], [], []
    for c in (1, 2):
        wshs.append(nc.dram_tensor(f"wsh{c}", [C, 9, C], F32R, kind="ExternalInput").ap())
        es_.append(nc.dram_tensor(f"e{c}", [C, 9, NPR + 1, 2, C], F8, kind="ExternalInput").ap())
        cbs_.append(nc.dram_tensor(f"cb{c}", [C, 1], F32, kind="ExternalInput").ap())
        btbls.append(nc.dram_tensor(f"btbl{c}", [9, C], F32R, kind="ExternalInput").ap())
    out_ap = nc.dram_tensor("out", [NSH, C, H, W], F32, kind="ExternalOutput").ap()
    if debug:
        dbg_p = nc.dram_tensor("dbg_p", [C, PLANE], F32, kind="ExternalOutput").ap()
        dbg_a = nc.dram_tensor("dbg_a", [C, NSH, H, W], F32, kind="ExternalOutput").ap()

    NCHUNK = NSH * 4
    NLOC = float(NSH * H * W)
    INV_N = 1.0 / (64 * H * W) if use_cc else 1.0 / NLOC
    GP = PLANE + 2              # flat plane with 1-elem guards at both ends

    with tile.TileContext(nc) as tc, contextlib.ExitStack() as ctx:
        const = ctx.enter_context(tc.tile_pool(name="const", bufs=1))
        planes = ctx.enter_context(tc.tile_pool(name="planes", bufs=1))
        lvlpool = ctx.enter_context(tc.tile_pool(name="lvlpool", bufs=2))
        scratch = ctx.enter_context(tc.tile_pool(name="scratch", bufs=1))
        small = ctx.enter_context(tc.tile_pool(name="small", bufs=4))
        psum = ctx.enter_context(tc.tile_pool(name="psum", bufs=8, space="PSUM"))
        dram = ctx.enter_context(tc.tile_pool(name="dram", bufs=4, space="DRAM"))

        # ---- constants ----
        wsh_t, e_t, cb_t, btbl_t = [], [], [], []
        for c in range(2):
            t = const.tile([C, 9, C], F32R, tag=f"wsh{c}")
            nc.sync.dma_start(out=t, in_=wshs[c])
            wsh_t.append(t)
            t = const.tile([C, 9, NPR + 1, 2, C], F8, tag=f"e{c}")
            nc.sync.dma_start(out=t, in_=es_[c])
            e_t.append(t)
            t = const.tile([C, 1], F32, tag=f"cb{c}")
            nc.sync.dma_start(out=t, in_=cbs_[c])
            cb_t.append(t)
            t = const.tile([9, C], F32R, tag=f"btbl{c}")
            nc.sync.dma_start(out=t, in_=btbls[c])
            btbl_t.append(t)
        mcls_t = const.tile([9, 4, 8 * WP], F32R, tag="mcls")
        nc.sync.dma_start(out=mcls_t, in_=mcls_ap)
        gb_t = const.tile([C, 4], F32, tag="gb")
        nc.sync.dma_start(out=gb_t, in_=gb_ap)
        lv_t = const.tile([C, 2 * NLEV], F32, tag="lv")
        nc.sync.dma_start(out=lv_t, in_=lv_ap)
        mones_t = const.tile([C, C], F32R, tag="mones")
        nc.vector.memset(mones_t[:].bitcast(F32), -1.0)
        nc.vector.tensor_copy(mones_t[:], mones_t[:])

        # ---- persistent planes (flat [C, GP], data at [1 : 1+PLANE]) ----
        xplane = planes.tile([C, GP], F32R, tag="xplane")
        pplane = planes.tile([C, GP], F32R, tag="pplane")
        absp = planes.tile([C, GP], F32R, tag="absp")
        relu2 = planes.tile([C, 2, PLANE], BF16, tag="relu2")
        a_t = planes.tile([C, NSH, H, W], F32, tag="a")
        # delta pair: slot0 = negind (rewritten per phase), slot1 = zero
        nislot = planes.tile([C, 2, GP], F8, tag="nislot")
        nc.vector.memset(nislot[:], 0.0)
        nc.vector.tensor_copy(nislot[:], nislot[:])
        for pl in (xplane, pplane, absp):
            nc.vector.memset(pl[:].bitcast(F32), 0.0)
            nc.vector.tensor_copy(pl[:], pl[:])

        def im(plane, li):
            """[C, 32, 34] row view of image li inside a flat [C, GP] plane."""
            return plane[:, 1 + li * (H * WP): 1 + (li + 1) * (H * WP)].rearrange(
                "p (h w) -> p h w", h=H)

        def win(plane, li, row, nrows, kw):
            """flat full-width window: rows [row, row+nrows) shifted by kw-1."""
            s = 1 + (li * H + row) * WP + (kw - 1)
            return plane[:, s: s + nrows * WP]

        def pwin(pair, li, row, nrows, kw):
            """same, for a [C, 2, GP] pair tile -> [C, 2, nrows*WP]."""
            s = 1 + (li * H + row) * WP + (kw - 1)
            return pair[:, :, s: s + nrows * WP]

        def mm(ps_ap, lhsT, rhs, first, last, pm=None):
            nc.tensor.matmul(ps_ap, lhsT, rhs, start=first, stop=last,
                             perf_mode=pm)

        def conv_phase(cidx):
            """shift conv: xplane -> pplane; zero pads make flat windows exact
            on real cols; junk cols 0/33 are never evacuated."""
            for li in range(HALF):
                for r0 in (0, 8, 16, 24):
                    ps = psum.tile([C, 8, WP], F32, tag="ps")
                    order = [4] + [t for t in range(9) if t != 4]
                    for i, tap in enumerate(order):
                        kh, kw = tap // 3, tap % 3
                        h0 = max(r0, 1 - kh) - r0
                        h1 = min(r0 + 8, 33 - kh) - r0
                        src = win(xplane, li, r0 + h0 + kh - 1, h1 - h0, kw)
                        mm(ps[:, h0:h1, :], wsh_t[cidx][:, tap, :], src,
                           i == 0, i == 8)
                    nc.scalar.activation(im(pplane, li)[:, r0:r0 + 8, 1:33],
                                         ps[:, :, 1:33], AT.Identity)

        def adder_phase(cidx, half, stats_t):
            """adder conv from pplane into a_t[half], stats col per chunk."""
            pfull = pplane[:, 1:1 + PLANE]
            # --- planes ---
            nc.scalar.activation(absp[:, 1:1 + PLANE], pfull, AT.Abs)
            nc.scalar.activation(relu2[:, 0], pfull, AT.Relu)
            nc.vector.tensor_scalar(out=relu2[:, 1], in0=pfull, scalar1=-1.0,
                                    scalar2=0.0, op0=OP.mult, op1=OP.max)
            # negind = min(relu(-p) * 2^12, tau): saturating indicator of p<0
            nc.vector.tensor_scalar(out=nislot[:, 0, 1:1 + PLANE],
                                    in0=relu2[:, 1], scalar1=float(2.0 ** 12),
                                    scalar2=TAU, op0=OP.mult, op1=OP.min)
            lvl = []
            for pr in range(NPR):
                lt = lvlpool.tile([C, 2, GP], F8, tag=f"lvl{pr}")
                lvl.append(lt)
                nc.vector.memset(lt[:, :, 0:1], 0.0)
                nc.vector.memset(lt[:, :, GP - 1:GP], 0.0)
                for hf in range(2):
                    sl = 2 * pr + hf
                    col = cidx * NLEV + sl
                    dst = lt[:, hf, 1:1 + PLANE]
                    if sl < LPS:      # pos: ACT +clamp = relu(|v| - relup)
                        nc.scalar.activation(dst, relu2[:, 0], AT.Relu,
                                             bias=lv_t[:, col:col + 1],
                                             scale=-1.0)
                    else:             # neg: DVE -clamp' = min(relunp - |v|, 0)
                        nc.vector.tensor_scalar(
                            out=dst, in0=relu2[:, 1],
                            scalar1=lv_t[:, col:col + 1], scalar2=0.0,
                            op0=OP.subtract, op1=OP.min)
            pairs = lvl + [nislot]
            # --- chunk matmuls ---
            for ci_, (li, r0) in enumerate([(a, b) for a in range(HALF)
                                            for b in (0, 8, 16, 24)]):
                ps = psum.tile([C, 8, WP], F32, tag="ps")
                par = r0 // 8
                # |p| conv (center tap first: full-chunk start)
                order = [4] + [t for t in range(9) if t != 4]
                for i, tap in enumerate(order):
                    kh, kw = tap // 3, tap % 3
                    h0 = max(r0, 1 - kh) - r0
                    h1 = min(r0 + 8, 33 - kh) - r0
                    src = win(absp, li, r0 + h0 + kh - 1, h1 - h0, kw)
                    mm(ps[:, h0:h1, :], mones_t[:], src, i == 0, False)
                # border class
                mm(ps[:], btbl_t[cidx], mcls_t[:, par, :], False, False)
                # DoubleRow pairs: NPR level pairs + (negind, 0) delta pair
                for tap in range(9):
                    kh, kw = tap // 3, tap % 3
                    h0 = max(r0, 1 - kh) - r0
                    h1 = min(r0 + 8, 33 - kh) - r0
                    for pr in range(NPR + 1):
                        src = pwin(pairs[pr], li, r0 + h0 + kh - 1, h1 - h0, kw)
                        last = (tap == 8) and (pr == NPR)
                        mm(ps[:, h0:h1, :], e_t[cidx][:, tap, pr], src,
                           False, last, pm=PM)
                # evac with bias + stats accumulation
                gi = half * 16 + ci_
                img = half * HALF + li
                nc.scalar.activation(a_t[:, img, r0:r0 + 8, :], ps[:, :, 1:33],
                                     AT.Identity, bias=cb_t[cidx][:],
                                     accum_out=stats_t[:, gi:gi + 1])

        def allreduce3(in_t):
            """AllReduce-add one [C,3] f32 SBUF tile across cores."""
            if not use_cc:
                return in_t
            ib = dram.tile([C, 3], F32, tag="arin")
            ob = dram.tile([C, 3], F32, tag="arout")
            nc.sync.dma_start(out=ib[:], in_=in_t[:])
            nc.gpsimd.collective_compute(
                "AllReduce", mybir.AluOpType.add,
                replica_groups=[list(range(NCORES))],
                ins=[ib.opt()], outs=[ob.opt()])
            rt = small.tile([C, 3], F32, tag="arres")
            nc.sync.dma_start(out=rt[:], in_=ob[:])
            return rt

        def bn_stats(stats_t):
            """local chunk sums -> (mu, rstd) via one [C,3] allreduce."""
            s_loc = small.tile([C, 1], F32, tag="sloc")
            nc.vector.tensor_reduce(s_loc[:], stats_t[:], mybir.AxisListType.X, OP.add)
            muloc = small.tile([C, 1], F32, tag="muloc")
            nc.vector.tensor_scalar(out=muloc[:], in0=s_loc[:], scalar1=1.0 / NLOC,
                                    scalar2=None, op0=OP.mult)
            nmuloc = small.tile([C, 1], F32, tag="nmuloc")
            nc.vector.tensor_scalar(out=nmuloc[:], in0=muloc[:], scalar1=-1.0,
                                    scalar2=None, op0=OP.mult)
            sq_t = small.tile([C, NSH], F32, tag="sqstats")
            for img in range(NSH):
                dumm = scratch.tile([C, H, W], F32, tag="fa")
                nc.scalar.activation(dumm[:], a_t[:, img, :, :], AT.Square,
                                     bias=nmuloc[:], accum_out=sq_t[:, img:img + 1])
            pack = small.tile([C, 3], F32, tag="pack")
            nc.vector.tensor_copy(pack[:, 0:1], s_loc[:])
            nc.vector.tensor_reduce(pack[:, 1:2], sq_t[:], mybir.AxisListType.X, OP.add)
            nc.vector.tensor_tensor(out=pack[:, 2:3], in0=muloc[:], in1=muloc[:],
                                    op=OP.mult)
            nc.vector.tensor_scalar(out=pack[:, 2:3], in0=pack[:, 2:3],
                                    scalar1=NLOC, scalar2=None, op0=OP.mult)
            glob = allreduce3(pack)
            mu = small.tile([C, 1], F32, tag="mu")
            nc.vector.tensor_scalar(out=mu[:], in0=glob[:, 0:1], scalar1=INV_N,
                                    scalar2=None, op0=OP.mult)
            # var = (s2 + s3)*INV_N - mu^2 + EPS
            var = small.tile([C, 1], F32, tag="var")
            nc.vector.tensor_tensor(out=var[:], in0=glob[:, 1:2], in1=glob[:, 2:3],
                                    op=OP.add)
            mu2 = small.tile([C, 1], F32, tag="mu2")
            nc.vector.tensor_tensor(out=mu2[:], in0=mu[:], in1=mu[:], op=OP.mult)
            nc.vector.tensor_scalar(out=var[:], in0=var[:], scalar1=INV_N,
                                    scalar2=EPS, op0=OP.mult, op1=OP.add)
            nc.vector.tensor_tensor(out=var[:], in0=var[:], in1=mu2[:], op=OP.subtract)
            sd = small.tile([C, 1], F32, tag="sd")
            nc.scalar.activation(sd[:], var[:], AT.Sqrt)
            rstd = small.tile([C, 1], F32, tag="rstd")
            nc.vector.reciprocal(rstd[:], sd[:])
            return mu, rstd

        def bn_coefs(mu, rstd, gcol, bcol):
            scale = small.tile([C, 1], F32, tag=f"scale{gcol}")
            nc.vector.tensor_scalar_mul(scale[:], rstd[:], gb_t[:, gcol:gcol + 1])
            nbias = small.tile([C, 1], F32, tag=f"nbias{gcol}")
            nc.vector.tensor_tensor(out=nbias[:], in0=mu[:], in1=scale[:], op=OP.mult)
            nc.vector.tensor_tensor(out=nbias[:], in0=gb_t[:, bcol:bcol + 1],
                                    in1=nbias[:], op=OP.subtract)
            return scale, nbias

        # =================== pipeline ===================
        stats1 = small.tile([C, NCHUNK], F32, tag="stats1")
        stats2 = small.tile([C, NCHUNK], F32, tag="stats2")

        # block 1: x -> conv1 -> adder1, both halves
        for half in range(2):
            for li in range(HALF):
                img = half * HALF + li
                xin = scratch.tile([C, H, W], F32, tag="xin")
                nc.sync.dma_start(out=xin[:], in_=x_ap[img])
                nc.vector.tensor_copy(im(xplane, li)[:, :, 1:33], xin[:])
            conv_phase(0)
            if debug and half == 0:
                nc.sync.dma_start(out=dbg_p, in_=pplane[:, 1:1 + PLANE].bitcast(F32))
            adder_phase(0, half, stats1)

        if debug:
            nc.sync.dma_start(out=dbg_a, in_=a_t[:])
        mu1, rstd1 = bn_stats(stats1)
        scale1, nbias1 = bn_coefs(mu1, rstd1, 0, 1)

        # block 2: relu(BN1(a1)) -> conv2 -> adder2
        for half in range(2):
            for li in range(HALF):
                img = half * HALF + li
                nc.scalar.activation(im(xplane, li)[:, :, 1:33], a_t[:, img, :, :],
                                     AT.Relu, bias=nbias1[:], scale=scale1[:])
            conv_phase(1)
            adder_phase(1, half, stats2)

        mu2, rstd2 = bn_stats(stats2)
        scale2, nbias2 = bn_coefs(mu2, rstd2, 2, 3)

        # out = relu(BN2(a2) + x), per image
        for img in range(NSH):
            t = scratch.tile([C, H, W], F32, tag="fa")
            nc.vector.tensor_scalar(out=t[:], in0=a_t[:, img, :, :],
                                    scalar1=scale2[:], scalar2=nbias2[:],
                                    op0=OP.mult, op1=OP.add)
            xin = scratch.tile([C, H, W], F32, tag="xin")
            nc.sync.dma_start(out=xin[:], in_=x_ap[img])
            nc.vector.tensor_tensor(out=t[:], in0=t[:], in1=xin[:], op=OP.add)
            nc.scalar.activation(xin[:], t[:], AT.Relu)
            nc.sync.dma_start(out=out_ap[img], in_=xin[:])

    nc.compile()
    return nc


def _bench_run(nc, in_maps, iters=5):
    """Times jitted multi-core executions (device-resident inputs)."""
    import time
    import jax
    from jax.sharding import Mesh, PartitionSpec, NamedSharding
    from jax.experimental.shard_map import shard_map
    from concourse import mybir
    from concourse.bass2jax import _bass_exec_p, install_neuronx_cc_hook, partition_id_tensor

    install_neuronx_cc_hook()
    n_cores = len(in_maps)
    in_names, out_names, out_avals, zero_outs = [], [], [], []
    for alloc in nc.m.functions[0].allocations:
        if not isinstance(alloc, mybir.MemoryLocationSet):
            continue
        name = alloc.memorylocations[0].name
        pid_name = nc.partition_id_tensor.name if nc.partition_id_tensor else None
        if alloc.kind == "ExternalInput":
            if name != pid_name:
                in_names.append(name)
        elif alloc.kind == "ExternalOutput":
            shape = tuple(alloc.tensor_shape)
            dtype = mybir.dt.np(alloc.dtype)
            out_names.append(name)
            out_avals.append(jax.core.ShapedArray(shape, dtype))
            zero_outs.append(np.zeros(shape, dtype))
    n_params = len(in_names)
    pid_name = nc.partition_id_tensor.name if nc.partition_id_tensor else None
    all_names = in_names + out_names + ([pid_name] if pid_name else [])

    def _body(*args):
        operands = list(args)
        if pid_name:
            operands.append(partition_id_tensor())
        outs = _bass_exec_p.bind(
            *operands, out_avals=tuple(out_avals), in_names=tuple(all_names),
            out_names=tuple(out_names), lowering_input_output_aliases=(),
            sim_require_finite=True, sim_require_nnan=True, nc=nc)
        return tuple(outs)

    devices = jax.devices()[:n_cores]
    mesh = Mesh(np.asarray(devices), ("core",))
    in_specs = (PartitionSpec("core"),) * (n_params + len(out_names))
    out_specs = (PartitionSpec("core"),) * len(out_names)
    fn = jax.jit(shard_map(_body, mesh=mesh, in_specs=in_specs,
                           out_specs=out_specs, check_rep=False))
    sh = NamedSharding(mesh, PartitionSpec("core"))
    args = [jax.device_put(
        np.concatenate([np.asarray(in_maps[c][nm]) for c in range(n_cores)], axis=0), sh)
        for nm in in_names]
    args += [jax.device_put(
        np.zeros((n_cores * z.shape[0], *z.shape[1:]), z.dtype), sh) for z in zero_outs]
    outs = fn(*args)
    jax.block_until_ready(outs)
    times = []
    for _ in range(iters):
        t0 = time.perf_counter()
        outs = fn(*args)
        jax.block_until_ready(outs)
        times.append(time.perf_counter() - t0)
    out_np = np.asarray(outs[0])
    per_core = np.split(out_np, n_cores, axis=0)
    results = [{out_names[0]: pc} for pc in per_core]
    return results, times


LAST_TIMES = None
LAST_RESULT = None


def kernel(**inputs):
    from concourse.bass_utils import run_bass_kernel_spmd

    x = np.ascontiguousarray(inputs["x"], np.float32)          # [64,128,32,32]
    key = ("prog",)
    if key not in _CACHE:
        _CACHE[key] = _build_program()
    nc = _CACHE[key]

    hkey = ("host",)
    if hkey not in _CACHE:
        h1 = _host_prep_adder(np.asarray(inputs["w_add1"], np.float64))
        h2 = _host_prep_adder(np.asarray(inputs["w_add2"], np.float64))
        gb = np.stack([np.asarray(inputs["gamma1"], np.float32),
                       np.asarray(inputs["beta1"], np.float32),
                       np.asarray(inputs["gamma2"], np.float32),
                       np.asarray(inputs["beta2"], np.float32)], axis=1)
        wsh1 = np.ascontiguousarray(
            np.asarray(inputs["w_shift1"], np.float32).reshape(C, C, 9).transpose(1, 2, 0))
        wsh2 = np.ascontiguousarray(
            np.asarray(inputs["w_shift2"], np.float32).reshape(C, C, 9).transpose(1, 2, 0))
        lv = np.zeros((C, 2 * NLEV), np.float32)
        for sl in range(NLEV):
            lv[:, sl] = abs(float(h1["levels"][sl]))
            lv[:, NLEV + sl] = abs(float(h2["levels"][sl]))
        shared = {
            "gb": gb, "mcls": _host_mcls(), "lv": lv,
            "wsh1": wsh1, "e1": h1["e"], "cb1": h1["cb"], "btbl1": h1["btbl"],
            "wsh2": wsh2, "e2": h2["e"], "cb2": h2["cb"], "btbl2": h2["btbl"],
        }
        _CACHE[hkey] = shared
    shared = _CACHE[hkey]

    in_maps = []
    for core in range(NCORES):
        m = dict(shared)
        m["x"] = np.ascontiguousarray(x[core * NSH:(core + 1) * NSH])
        in_maps.append(m)

    import os
    global LAST_RESULT, LAST_TIMES
    if os.environ.get("BASICBLOCK_BENCH", "0") == "1":
        results, times = _bench_run(nc, in_maps, iters=int(os.environ.get("BENCH_ITERS", "5")))
        LAST_TIMES = times
        LAST_RESULT = None
        return np.concatenate([r["out"] for r in results], axis=0)
    res = run_bass_kernel_spmd(nc, in_maps, core_ids=list(range(NCORES)))
    LAST_RESULT = res
    out = np.concatenate([r["out"] for r in res.results], axis=0)
    return out


# revision 36
# speedup vs baseline: 1.2747x; 1.2747x over previous
"""Trainium2 Bass kernel for nn_BasicBlock (ShiftNet+AdderNet block), data-parallel on 8 cores.

Adder conv decomposition (per valid (ci,tap) term, exact up to level approx):
  -|p-w| = -|p| + cK + 2*delta*1[p<0] - 2*clamp_v(p)
where v = v(w) is one of 6 global value levels (1D k-means per sign, snapped
to the fp8e4 grid), delta = v - w, cK = w + 2|v|*1[w<0], and
  clamp_v(p) = relu(v - relu(p))       (v > 0)
  clamp'_v(p) = relu(|v| - relu(-p))   (v < 0)
Planes are tap-independent: per phase we build |p| (f32r), negind = tau*1[p<0]
(fp8), and 6 clamp planes (fp8). The level matmuls run as fp8 DoubleRow pairs
(two 128-deep contraction planes per instruction at 0.5 cyc/row); the delta
conv rides (negind, zero) as a 4th pair; |p| uses an all-(-1) f32r matmul.
Borders: H-pad taps are skipped by matmul row restriction; W-pad columns hold
plane(0) values; both corrected by a 9-class border table matmul.
BN uses one [C,3] AllReduce per BN (parallel variance combine).
"""
import numpy as np
import ml_dtypes

NCORES = 8
NSH = 8            # images per core
HALF = 4           # images per half-pass
H = W = 32
C = 128
WP = 34            # w-padded plane width
PLANE = HALF * H * WP
EPS = 1e-5
TAU = 2.0 ** -8
LPS = 1            # levels per sign
NLEV = 2 * LPS
NPR = NLEV // 2    # level pairs (the delta conv adds one more pair)

_CACHE = {}
F8NP = ml_dtypes.float8_e4m3


def _f8(x):
    return np.asarray(x, np.float64).astype(F8NP).astype(np.float64)


def _kmeans1d(vals, k, iters=50):
    cent = np.quantile(vals, (np.arange(k) + 0.5) / k)
    for _ in range(iters):
        idx = np.abs(vals[:, None] - cent[None, :]).argmin(1)
        for j in range(k):
            m = idx == j
            if m.any():
                cent[j] = vals[m].mean()
        cent = np.sort(cent)
    cent = np.unique(_f8(cent))
    if len(cent) < k:
        cent = np.concatenate([cent, np.repeat(cent[-1], k - len(cent))])
    return cent


def _host_prep_adder(wa64):
    """wa64: [co, ci, 3, 3] float64. Device arrays for one adder conv."""
    co_n = wa64.shape[0]
    wk = wa64.reshape(co_n, C, 9)                    # [co, ci, tap]
    assert (wk != 0.0).all(), "zero adder weight breaks sign split"
    lev_p = _kmeans1d(wk[wk > 0], LPS)               # ascending, > 0
    lev_n = -_kmeans1d(-wk[wk < 0], LPS)[::-1]       # ascending, < 0
    levels = np.concatenate([lev_p, lev_n])          # slots 0..2 pos, 3..5 neg
    ip = np.abs(wk[..., None] - lev_p[None, None, None, :]).argmin(-1)
    inn = np.abs(wk[..., None] - lev_n[None, None, None, :]).argmin(-1)
    pos = wk > 0
    assign = np.where(pos, ip, LPS + inn)            # [co, ci, tap]
    v = np.where(pos, lev_p[ip], lev_n[inn])
    delta = v - wk
    cK = wk + 2.0 * np.abs(v) * (~pos)
    cb = cK.sum(axis=(1, 2)).reshape(co_n, 1)        # [co, 1]
    # lhsT pairs: [ci, tap, pair, half, co] fp8; all clamp planes are computed
    # as -clamp on DVE, so every level slot scatters with +2.
    #   pair NPR: half0 = delta kernel 2*delta/TAU on negind, half1 = 0
    E = np.zeros((C, 9, NPR + 1, 2, co_n), np.float64)
    for lvl in range(NLEV):
        sgn = 2.0
        pr, hf = lvl // 2, lvl % 2
        for tap in range(9):
            sel = assign[:, :, tap] == lvl           # [co, ci]
            E[:, tap, pr, hf, :][sel.T] = sgn
    for tap in range(9):
        E[:, tap, NPR, 0, :] = (2.0 / TAU) * delta[:, :, tap].T
    # border table: 9 classes [9, co]
    absw = np.abs(wk)
    hcorr = -(absw + cK).sum(axis=1)                 # [co, tap]
    wcorr = (2.0 * delta * pos).sum(axis=1)          # [co, tap]
    btbl = np.zeros((9, co_n), np.float64)
    for hcls in range(3):
        for wcls in range(3):
            cls = hcls * 3 + wcls
            for tap in range(9):
                kh, kw = tap // 3, tap % 3
                h_pad = (hcls == 0 and kh == 0) or (hcls == 2 and kh == 2)
                w_pad = (wcls == 0 and kw == 0) or (wcls == 2 and kw == 2)
                if h_pad:
                    btbl[cls] += hcorr[:, tap]
                elif w_pad:
                    btbl[cls] += wcorr[:, tap]
    return dict(levels=levels.astype(np.float32),
                e=np.ascontiguousarray(E.astype(F8NP)),
                cb=cb.astype(np.float32), btbl=btbl.astype(np.float32))


CHUNKS = [(0, 11), (11, 11), (22, 10)]   # (row0, nrows) per image


def _host_mcls():
    """class-indicator rhs [9 cls, 3 parity, 11*34] (f32); cols 0/33 junk -> 0."""
    m = np.zeros((9, 3, 11, WP), np.float32)
    for par, (r0, nr) in enumerate(CHUNKS):
        for hr in range(nr):
            h = r0 + hr
            hcls = 0 if h == 0 else (2 if h == 31 else 1)
            for w in range(W):
                wcls = 0 if w == 0 else (2 if w == 31 else 1)
                m[hcls * 3 + wcls, par, hr, w + 1] = 1.0
    return np.ascontiguousarray(m.reshape(9, 3, 11 * WP))


def _build_program(use_cc=True, debug=False):
    import concourse.bass as bass
    import concourse.bacc as bacc
    import concourse.tile as tile
    import contextlib
    from concourse import mybir

    F32 = mybir.dt.float32
    F32R = mybir.dt.float32r
    BF16 = mybir.dt.bfloat16
    F8 = mybir.dt.float8e4
    AT = mybir.ActivationFunctionType
    OP = mybir.AluOpType
    PM = mybir.MatmulPerfMode.DoubleRow

    nc = bacc.Bacc("TRN2", target_bir_lowering=False, debug=False,
                   num_devices=NCORES if use_cc else 1)

    x_ap = nc.dram_tensor("x", [NSH, C, H, W], F32, kind="ExternalInput").ap()
    gb_ap = nc.dram_tensor("gb", [C, 4], F32, kind="ExternalInput").ap()
    mcls_ap = nc.dram_tensor("mcls", [9, 3, 11 * WP], F32R, kind="ExternalInput").ap()
    lv_ap = nc.dram_tensor("lv", [C, 2 * NLEV], F32, kind="ExternalInput").ap()
    wshs, es_, cbs_, btbls = [], [], [], []
    for c in (1, 2):
        wshs.append(nc.dram_tensor(f"wsh{c}", [C, 9, C], F32R, kind="ExternalInput").ap())
        es_.append(nc.dram_tensor(f"e{c}", [C, 9, NPR + 1, 2, C], F8, kind="ExternalInput").ap())
        cbs_.append(nc.dram_tensor(f"cb{c}", [C, 1], F32, kind="ExternalInput").ap())
        btbls.append(nc.dram_tensor(f"btbl{c}", [9, C], F32R, kind="ExternalInput").ap())
    out_ap = nc.dram_tensor("out", [NSH, C, H, W], F32, kind="ExternalOutput").ap()

    NCHUNK = NSH * len(CHUNKS)
    NLOC = float(NSH * H * W)
    INV_N = 1.0 / (64 * H * W) if use_cc else 1.0 / NLOC
    GP = PLANE + 2              # flat plane with 1-elem guards at both ends

    with tile.TileContext(nc) as tc, contextlib.ExitStack() as ctx:
        const = ctx.enter_context(tc.tile_pool(name="const", bufs=1))
        planes = ctx.enter_context(tc.tile_pool(name="planes", bufs=1))
        lvlpool = ctx.enter_context(tc.tile_pool(name="lvlpool", bufs=2))
        scratch = ctx.enter_context(tc.tile_pool(name="scratch", bufs=2))
        small = ctx.enter_context(tc.tile_pool(name="small", bufs=4))
        psum = ctx.enter_context(tc.tile_pool(name="psum", bufs=8, space="PSUM"))
        dram = ctx.enter_context(tc.tile_pool(name="dram", bufs=4, space="DRAM"))

        # ---- constants (block-1 consts first; block-2 loads emitted after
        # the block-1 instruction stream so the x DMAs go out early) ----
        wsh_t, e_t, cb_t, btbl_t = [], [], [], []
        for c in range(2):
            t_wsh = const.tile([C, 9, C], F32R, tag=f"wsh{c}")
            wsh_t.append(t_wsh)
            t_e = const.tile([C, 9, NPR + 1, 2, C], F8, tag=f"e{c}")
            e_t.append(t_e)
            t_cb = const.tile([C, 1], F32, tag=f"cb{c}")
            cb_t.append(t_cb)
            t_btbl = const.tile([9, C], F32R, tag=f"btbl{c}")
            btbl_t.append(t_btbl)

        def load_consts(c, defer_e=False):
            nc.sync.dma_start(out=wsh_t[c], in_=wshs[c])
            if not defer_e:
                nc.sync.dma_start(out=e_t[c], in_=es_[c])
            nc.sync.dma_start(out=cb_t[c], in_=cbs_[c])
            nc.sync.dma_start(out=btbl_t[c], in_=btbls[c])

        load_consts(0, defer_e=True)
        mcls_t = const.tile([9, 3, 11 * WP], F32R, tag="mcls")
        nc.sync.dma_start(out=mcls_t, in_=mcls_ap)
        gb_t = const.tile([C, 4], F32, tag="gb")
        nc.sync.dma_start(out=gb_t, in_=gb_ap)
        lv_t = const.tile([C, 2 * NLEV], F32, tag="lv")
        nc.sync.dma_start(out=lv_t, in_=lv_ap)
        mones_t = const.tile([C, C], F32R, tag="mones")
        nc.vector.memset(mones_t[:].bitcast(F32), -1.0)
        nc.vector.tensor_copy(mones_t[:], mones_t[:])

        # ---- persistent planes (flat [C, GP], data at [1 : 1+PLANE]) ----
        xplane = planes.tile([C, GP], F32R, tag="xplane")
        ppool = ctx.enter_context(tc.tile_pool(name="ppool", bufs=2))
        absp = planes.tile([C, GP], F32R, tag="absp")
        relu2 = planes.tile([C, 2, PLANE], BF16, tag="relu2")
        a_t = planes.tile([C, NSH, H, W], F32, tag="a")
        # delta pair: slot0 = negind (rewritten per phase), slot1 = zero
        nislot = planes.tile([C, 2, GP], F8, tag="nislot")

        def zero_pads(pl):
            """zero the W-pad column pairs + guards of a flat [C, GP] plane."""
            rows = pl[:, 1:1 + PLANE].rearrange("p (r w) -> p r w", w=WP)
            for view in (rows[:, :, 0:1], rows[:, :, WP - 1:WP],
                         pl[:, 0:1], pl[:, GP - 1:GP]):
                nc.vector.memset(view.bitcast(F32), 0.0)
                nc.vector.tensor_copy(view, view)

        def im(plane, li):
            """[C, 32, 34] row view of image li inside a flat [C, GP] plane."""
            return plane[:, 1 + li * (H * WP): 1 + (li + 1) * (H * WP)].rearrange(
                "p (h w) -> p h w", h=H)

        def win(plane, li, row, nrows, kw):
            """flat full-width window: rows [row, row+nrows) shifted by kw-1."""
            s = 1 + (li * H + row) * WP + (kw - 1)
            return plane[:, s: s + nrows * WP]

        def pwin(pair, li, row, nrows, kw):
            """same, for a [C, 2, GP] pair tile -> [C, 2, nrows*WP]."""
            s = 1 + (li * H + row) * WP + (kw - 1)
            return pair[:, :, s: s + nrows * WP]

        zero_pads(xplane)
        for view in (absp[:, 0:1], absp[:, GP - 1:GP]):
            nc.vector.memset(view.bitcast(F32), 0.0)
            nc.vector.tensor_copy(view, view)

        def mm(ps_ap, lhsT, rhs, first, last, pm=None):
            nc.tensor.matmul(ps_ap, lhsT, rhs, start=first, stop=last,
                             perf_mode=pm)

        def conv_phase(cidx, pplane, load_fn=None, half=0):
            """shift conv: xplane -> pplane; zero pads make flat windows exact
            on real cols; junk cols 0/33 are never evacuated. Per-image loads
            are interleaved so chunk 0 starts as soon as image 0 landed."""
            for li in range(HALF):
                if load_fn is not None:
                    load_fn(half * HALF + li, li)
                for r0, nr in CHUNKS:
                    ps = psum.tile([C, 11, WP], F32, tag="ps")
                    order = [4] + [t for t in range(9) if t != 4]
                    for i, tap in enumerate(order):
                        kh, kw = tap // 3, tap % 3
                        h0 = max(r0, 1 - kh) - r0
                        h1 = min(r0 + nr, 33 - kh) - r0
                        src = win(xplane, li, r0 + h0 + kh - 1, h1 - h0, kw)
                        mm(ps[:, h0:h1, :], wsh_t[cidx][:, tap, :], src,
                           i == 0, i == 8)
                    nc.scalar.activation(im(pplane, li)[:, r0:r0 + nr, 1:33],
                                         ps[:, :nr, 1:33], AT.Identity)

        def adder_phase(cidx, half, stats_t, pplane, sq_t, hsml, hnmu):
            """adder conv from pplane into a_t[half], stats col per chunk."""
            pfull = pplane[:, 1:1 + PLANE]
            # --- planes ---
            nc.scalar.activation(absp[:, 1:1 + PLANE], pfull, AT.Abs)
            nc.vector.tensor_scalar(out=relu2[:, 0], in0=pfull, scalar1=0.0,
                                    scalar2=None, op0=OP.max)
            nc.vector.tensor_scalar(out=relu2[:, 1], in0=pfull, scalar1=-1.0,
                                    scalar2=0.0, op0=OP.mult, op1=OP.max)
            # negind = min(relu(-p) * 2^12, tau): saturating indicator of p<0
            nc.vector.tensor_scalar(out=nislot[:, 0, 1:1 + PLANE],
                                    in0=relu2[:, 1], scalar1=float(2.0 ** 12),
                                    scalar2=TAU, op0=OP.mult, op1=OP.min)
            lvl = []
            for pr in range(NPR):
                lt = lvlpool.tile([C, 2, GP], F8, tag=f"lvl{pr}")
                lvl.append(lt)
                nc.vector.memset(lt[:, :, 0:1], 0.0)
                nc.vector.memset(lt[:, :, GP - 1:GP], 0.0)
                for hf in range(2):
                    sl = 2 * pr + hf
                    col = cidx * NLEV + sl
                    dst = lt[:, hf, 1:1 + PLANE]
                    # -clamp = min(relu_side - |v|, 0); level slots scatter +2
                    nc.vector.tensor_scalar(
                        out=dst, in0=relu2[:, 0 if sl < LPS else 1],
                        scalar1=lv_t[:, col:col + 1], scalar2=0.0,
                        op0=OP.subtract, op1=OP.min)
            pairs = lvl + [nislot]
            # --- chunk matmuls ---
            for ci_, (li, (r0, nr)) in enumerate([(a, b) for a in range(HALF)
                                                  for b in CHUNKS]):
                ps = psum.tile([C, 11, WP], F32, tag="ps")
                par = 0 if r0 == 0 else (1 if r0 == 11 else 2)
                order = [4] + [t for t in range(9) if t != 4]
                # DoubleRow pairs first (planes ready before absp); center tap
                # of pair 0 opens the accumulation group with full coverage
                for i, tap in enumerate(order):
                    kh, kw = tap // 3, tap % 3
                    h0 = max(r0, 1 - kh) - r0
                    h1 = min(r0 + nr, 33 - kh) - r0
                    for pr in range(NPR + 1):
                        src = pwin(pairs[pr], li, r0 + h0 + kh - 1, h1 - h0, kw)
                        mm(ps[:, h0:h1, :], e_t[cidx][:, tap, pr], src,
                           i == 0 and pr == 0, False, pm=PM)
                # border class
                mm(ps[:, :nr, :], btbl_t[cidx], mcls_t[:, par, :nr * WP],
                   False, False)
                # |p| conv last (gives the ACT Abs time to finish)
                for i, tap in enumerate(order):
                    kh, kw = tap // 3, tap % 3
                    h0 = max(r0, 1 - kh) - r0
                    h1 = min(r0 + nr, 33 - kh) - r0
                    src = win(absp, li, r0 + h0 + kh - 1, h1 - h0, kw)
                    mm(ps[:, h0:h1, :], mones_t[:], src, False, i == 8)
                # evac with bias + stats accumulation
                gi = half * (HALF * len(CHUNKS)) + ci_
                img = half * HALF + li
                nc.scalar.activation(a_t[:, img, r0:r0 + nr, :], ps[:, :nr, 1:33],
                                     AT.Identity, bias=cb_t[cidx][:],
                                     accum_out=stats_t[:, gi:gi + 1])
            # per-half partial BN stats (overlaps the other half's matmuls)
            nch = HALF * len(CHUNKS)
            dumm12 = small.tile([C, nch], F32, tag="dumm12")
            nc.scalar.activation(dumm12[:], stats_t[:, half * nch:(half + 1) * nch],
                                 AT.Identity, accum_out=hsml[:, half:half + 1])
            nc.vector.tensor_scalar(out=hnmu[:, half:half + 1],
                                    in0=hsml[:, half:half + 1],
                                    scalar1=-2.0 / NLOC, scalar2=None, op0=OP.mult)
            for q in range(2):
                # dummy target: absp data region is fully rewritten by the next
                # phase's Abs (pads included), so scribbling here is safe
                dumm = absp[:, 1:1 + 2 * H * W].rearrange(
                    "p (a b c) -> p a b c", a=2, b=H)
                i0 = half * HALF + 2 * q
                nc.scalar.activation(dumm, a_t[:, i0:i0 + 2, :, :], AT.Square,
                                     bias=hnmu[:, half:half + 1],
                                     accum_out=sq_t[:, 2 * half + q:2 * half + q + 1])

        def allreduce3(in_t):
            """AllReduce-add one [C,3] f32 SBUF tile across cores."""
            if not use_cc:
                return in_t
            ib = dram.tile([C, 3], F32, tag="arin")
            ob = dram.tile([C, 3], F32, tag="arout")
            nc.sync.dma_start(out=ib[:], in_=in_t[:])
            nc.gpsimd.collective_compute(
                "AllReduce", mybir.AluOpType.add,
                replica_groups=[list(range(NCORES))],
                ins=[ib.opt()], outs=[ob.opt()])
            rt = small.tile([C, 3], F32, tag="arres")
            nc.sync.dma_start(out=rt[:], in_=ob[:])
            return rt

        def bn_stats(sq_t, hsml, hnmu):
            """combine per-half partials -> (mu, rstd) via one [C,3] allreduce.
            Parallel variance over (core, half) groups:
              var = [sum_h ss_h + sum_h n_h*mu_h^2]/N - mu^2."""
            pack = small.tile([C, 3], F32, tag="pack")
            nc.vector.tensor_tensor(out=pack[:, 0:1], in0=hsml[:, 0:1],
                                    in1=hsml[:, 1:2], op=OP.add)
            dumm4 = small.tile([C, 4], F32, tag="dumm4")
            nc.scalar.activation(dumm4[:], sq_t[:], AT.Identity,
                                 accum_out=pack[:, 1:2])
            mu2h = small.tile([C, 2], F32, tag="mu2h")
            nc.vector.tensor_tensor(out=mu2h[:], in0=hnmu[:], in1=hnmu[:],
                                    op=OP.mult)
            dumm2 = small.tile([C, 2], F32, tag="dumm2")
            nc.scalar.activation(dumm2[:], mu2h[:], AT.Identity,
                                 accum_out=pack[:, 2:3])
            nc.vector.tensor_scalar(out=pack[:, 2:3], in0=pack[:, 2:3],
                                    scalar1=NLOC / 2.0, scalar2=None, op0=OP.mult)
            glob = allreduce3(pack)
            mu = small.tile([C, 1], F32, tag="mu")
            nc.vector.tensor_scalar(out=mu[:], in0=glob[:, 0:1], scalar1=INV_N,
                                    scalar2=None, op0=OP.mult)
            # var = (s2 + s3)*INV_N - mu^2 + EPS
            var = small.tile([C, 1], F32, tag="var")
            nc.vector.tensor_tensor(out=var[:], in0=glob[:, 1:2], in1=glob[:, 2:3],
                                    op=OP.add)
            mu2 = small.tile([C, 1], F32, tag="mu2")
            nc.vector.tensor_tensor(out=mu2[:], in0=mu[:], in1=mu[:], op=OP.mult)
            nc.vector.tensor_scalar(out=var[:], in0=var[:], scalar1=INV_N,
                                    scalar2=EPS, op0=OP.mult, op1=OP.add)
            nc.vector.tensor_tensor(out=var[:], in0=var[:], in1=mu2[:], op=OP.subtract)
            sd = small.tile([C, 1], F32, tag="sd")
            nc.scalar.activation(sd[:], var[:], AT.Sqrt)
            rstd = small.tile([C, 1], F32, tag="rstd")
            nc.vector.reciprocal_approx_fast(rstd[:], sd[:])
            return mu, rstd

        def bn_coefs(mu, rstd, gcol, bcol):
            scale = small.tile([C, 1], F32, tag=f"scale{gcol}")
            nc.vector.tensor_scalar_mul(scale[:], rstd[:], gb_t[:, gcol:gcol + 1])
            nbias = small.tile([C, 1], F32, tag=f"nbias{gcol}")
            nc.vector.tensor_tensor(out=nbias[:], in0=mu[:], in1=scale[:], op=OP.mult)
            nc.vector.tensor_tensor(out=nbias[:], in0=gb_t[:, bcol:bcol + 1],
                                    in1=nbias[:], op=OP.subtract)
            return scale, nbias

        # =================== pipeline ===================
        stats1 = small.tile([C, NCHUNK], F32, tag="stats1")
        stats2 = small.tile([C, NCHUNK], F32, tag="stats2")

        def block(cidx, stats_t, load_fn, post_conv=None):
            """conv both halves into double-buffered pplanes, then both adders;
            conv(h1) overlaps adder-plane generation of h0. Per-half BN partial
            stats are emitted inside adder_phase and overlap the other half."""
            pps = []
            for half in range(2):
                pp = ppool.tile([C, GP], F32R, tag="pp")
                zero_pads(pp)
                conv_phase(cidx, pp, load_fn=load_fn, half=half)
                if half == 0 and post_conv is not None:
                    post_conv()
                pps.append(pp)
            sq_t = small.tile([C, 4], F32, tag="sqstats")
            hsml = small.tile([C, 2], F32, tag="hsml")
            hnmu = small.tile([C, 2], F32, tag="hnmu")
            for half in range(2):
                adder_phase(cidx, half, stats_t, pps[half], sq_t, hsml, hnmu)
            return sq_t, hsml, hnmu

        def load_x(img, li):
            xin = scratch.tile([C, H, W], F32, tag="xin")
            eng = nc.scalar if img < HALF else nc.sync
            eng.dma_start(out=xin[:], in_=x_ap[img])
            nc.vector.tensor_copy(im(xplane, li)[:, :, 1:33], xin[:])

        def post_conv0():
            nc.sync.dma_start(out=e_t[0], in_=es_[0])
            # zero slot1 + guards of the delta pair (kept zero for the whole
            # kernel); deferred here so it doesn't head the DVE queue
            nc.vector.memset(nislot[:, 1], 0.0)
            nc.vector.tensor_copy(nislot[:, 1], nislot[:, 1])
            for v in (nislot[:, 0, 0:1], nislot[:, 0, GP - 1:GP]):
                nc.vector.memset(v, 0.0)
                nc.vector.tensor_copy(v, v)

        st1 = block(0, stats1, load_x, post_conv=post_conv0)
        load_consts(1)
        mu1, rstd1 = bn_stats(*st1)
        scale1, nbias1 = bn_coefs(mu1, rstd1, 0, 1)

        def load_bn(img, li):
            nc.scalar.activation(im(xplane, li)[:, :, 1:33], a_t[:, img, :, :],
                                 AT.Relu, bias=nbias1[:], scale=scale1[:])

        st2 = block(1, stats2, load_bn)

        xins = []
        for b in range(2):
            i0 = 2 * b
            xin = scratch.tile([C, 2, H, W], F32, tag="xin")
            nc.scalar.dma_start(out=xin[:],
                                in_=x_ap[i0:i0 + 2].rearrange("n c h w -> c n h w"))
            xins.append(xin)

        mu2, rstd2 = bn_stats(*st2)
        scale2, nbias2 = bn_coefs(mu2, rstd2, 2, 3)

        # out = relu(BN2(a2) + x), 2-image pipelined batches, in place on a_t
        for b in range(NSH // 2):
            i0 = 2 * b
            av = a_t[:, i0:i0 + 2, :, :]
            if b < 2:
                xin = xins[b]
            else:
                xin = scratch.tile([C, 2, H, W], F32, tag="xin")
                nc.scalar.dma_start(out=xin[:],
                                    in_=x_ap[i0:i0 + 2].rearrange("n c h w -> c n h w"))
            nc.vector.tensor_scalar(out=av, in0=av, scalar1=scale2[:],
                                    scalar2=nbias2[:], op0=OP.mult, op1=OP.add)
            nc.vector.tensor_tensor(out=av, in0=av, in1=xin[:], op=OP.add)
            nc.scalar.activation(xin[:], av, AT.Relu)
            nc.sync.dma_start(out=out_ap[i0:i0 + 2].rearrange("n c h w -> c n h w"),
                              in_=xin[:])

    nc.compile()
    return nc


def _bench_run(nc, in_maps, iters=5):
    """Times jitted multi-core executions (device-resident inputs)."""
    import time
    import jax
    from jax.sharding import Mesh, PartitionSpec, NamedSharding
    from jax.experimental.shard_map import shard_map
    from concourse import mybir
    from concourse.bass2jax import _bass_exec_p, install_neuronx_cc_hook, partition_id_tensor

    install_neuronx_cc_hook()
    n_cores = len(in_maps)
    in_names, out_names, out_avals, zero_outs = [], [], [], []
    for alloc in nc.m.functions[0].allocations:
        if not isinstance(alloc, mybir.MemoryLocationSet):
            continue
        name = alloc.memorylocations[0].name
        pid_name = nc.partition_id_tensor.name if nc.partition_id_tensor else None
        if alloc.kind == "ExternalInput":
            if name != pid_name:
                in_names.append(name)
        elif alloc.kind == "ExternalOutput":
            shape = tuple(alloc.tensor_shape)
            dtype = mybir.dt.np(alloc.dtype)
            out_names.append(name)
            out_avals.append(jax.core.ShapedArray(shape, dtype))
            zero_outs.append(np.zeros(shape, dtype))
    n_params = len(in_names)
    pid_name = nc.partition_id_tensor.name if nc.partition_id_tensor else None
    all_names = in_names + out_names + ([pid_name] if pid_name else [])

    def _body(*args):
        operands = list(args)
        if pid_name:
            operands.append(partition_id_tensor())
        outs = _bass_exec_p.bind(
            *operands, out_avals=tuple(out_avals), in_names=tuple(all_names),
            out_names=tuple(out_names), lowering_input_output_aliases=(),
            sim_require_finite=True, sim_require_nnan=True, nc=nc)
        return tuple(outs)

    devices = jax.devices()[:n_cores]
    mesh = Mesh(np.asarray(devices), ("core",))
    in_specs = (PartitionSpec("core"),) * (n_params + len(out_names))
    out_specs = (PartitionSpec("core"),) * len(out_names)
    fn = jax.jit(shard_map(_body, mesh=mesh, in_specs=in_specs,
                           out_specs=out_specs, check_rep=False))
    sh = NamedSharding(mesh, PartitionSpec("core"))
    args = [jax.device_put(
        np.concatenate([np.asarray(in_maps[c][nm]) for c in range(n_cores)], axis=0), sh)
        for nm in in_names]
    args += [jax.device_put(
        np.zeros((n_cores * z.shape[0], *z.shape[1:]), z.dtype), sh) for z in zero_outs]
    outs = fn(*args)
    jax.block_until_ready(outs)
    times = []
    for _ in range(iters):
        t0 = time.perf_counter()
        outs = fn(*args)
        jax.block_until_ready(outs)
        times.append(time.perf_counter() - t0)
    out_np = np.asarray(outs[0])
    per_core = np.split(out_np, n_cores, axis=0)
    results = [{out_names[0]: pc} for pc in per_core]
    return results, times


LAST_TIMES = None
LAST_RESULT = None


def kernel(**inputs):
    from concourse.bass_utils import run_bass_kernel_spmd

    x = np.ascontiguousarray(inputs["x"], np.float32)          # [64,128,32,32]
    key = ("prog",)
    if key not in _CACHE:
        _CACHE[key] = _build_program()
    nc = _CACHE[key]

    hkey = ("host",)
    if hkey not in _CACHE:
        h1 = _host_prep_adder(np.asarray(inputs["w_add1"], np.float64))
        h2 = _host_prep_adder(np.asarray(inputs["w_add2"], np.float64))
        gb = np.stack([np.asarray(inputs["gamma1"], np.float32),
                       np.asarray(inputs["beta1"], np.float32),
                       np.asarray(inputs["gamma2"], np.float32),
                       np.asarray(inputs["beta2"], np.float32)], axis=1)
        wsh1 = np.ascontiguousarray(
            np.asarray(inputs["w_shift1"], np.float32).reshape(C, C, 9).transpose(1, 2, 0))
        wsh2 = np.ascontiguousarray(
            np.asarray(inputs["w_shift2"], np.float32).reshape(C, C, 9).transpose(1, 2, 0))
        lv = np.zeros((C, 2 * NLEV), np.float32)
        for sl in range(NLEV):
            lv[:, sl] = abs(float(h1["levels"][sl]))
            lv[:, NLEV + sl] = abs(float(h2["levels"][sl]))
        shared = {
            "gb": gb, "mcls": _host_mcls(), "lv": lv,
            "wsh1": wsh1, "e1": h1["e"], "cb1": h1["cb"], "btbl1": h1["btbl"],
            "wsh2": wsh2, "e2": h2["e"], "cb2": h2["cb"], "btbl2": h2["btbl"],
        }
        _CACHE[hkey] = shared
    shared = _CACHE[hkey]

    in_maps = []
    for core in range(NCORES):
        m = dict(shared)
        m["x"] = np.ascontiguousarray(x[core * NSH:(core + 1) * NSH])
        in_maps.append(m)

    import os
    global LAST_RESULT, LAST_TIMES
    if os.environ.get("BASICBLOCK_BENCH", "0") == "1":
        results, times = _bench_run(nc, in_maps, iters=int(os.environ.get("BENCH_ITERS", "5")))
        LAST_TIMES = times
        LAST_RESULT = None
        return np.concatenate([r["out"] for r in results], axis=0)
    res = run_bass_kernel_spmd(nc, in_maps, core_ids=list(range(NCORES)))
    LAST_RESULT = res
    out = np.concatenate([r["out"] for r in res.results], axis=0)
    return out
